# revision 1
# baseline (speedup 1.0000x reference)
"""Trainium2 Bass kernel for nn_Encoder (4-layer dense transformer encoder).

Sharding: sequence-sharded data parallel. 8 cores = 2 batches x 4 sequence
chunks of 256 tokens. Per layer each core computes its own K chunk (both the
[HD, keys] and [keys, HD] layouts) and AllGathers K within its batch's
4-core group. Activations stay transposed (xT [D, L_local]) so every matmul
contracts over the partition dim. All matmul inputs are float32r (fp32 bits;
the PE rounds to 11 explicit mantissa bits and runs at full rate for N>=256).

The attention key mask is applied multiplicatively: rows of k_nat for masked
keys are zeroed and the softmax denominator uses the 0/1 keep-mask as the
stationary column, which is numerically identical to the reference's
where(mask, -1e9, score) followed by softmax.

Self-contained: hardcodes all shapes; host side does the embedding gather,
positional encoding, weight permutations/reshapes, and output assembly.
"""
import os
import numpy as np

B, L, D, H, M, NL, V = 2, 1024, 1024, 16, 4096, 4, 32000
HD = D // H          # 64
LC = 256             # local sequence chunk per core
NCORES = 8
PAD = 0

_DEV_NL = int(os.environ.get("KERNEL_NL", str(NL)))

_cache = {}
PHASE_LOG = []


def _build_nc(n_layers, reps=1):
    import os
    import contextlib
    import concourse.mybir as mybir
    import concourse.tile as tile
    from concourse import bacc
    from concourse.masks import make_identity

    f32 = mybir.dt.float32
    f32r = mybir.dt.float32r
    AF = mybir.ActivationFunctionType
    ALU = mybir.AluOpType

    FAKE_AG = os.environ.get("KERNEL_FAKE_AG", "0") == "1"
    nc = bacc.Bacc(None, target_bir_lowering=False, num_devices=NCORES)
    PHASE_LOG.clear()

    def mark(label):
        PHASE_LOG.append((label, len(nc.inst_map)))

    def par(name, shape, dt, out=False):
        return nc.declare_dram_parameter(name, list(shape), dt, isOutput=out)

    xT_in = par("xT_in", [8, 128, LC], f32r)
    wq_in = par("wq", [n_layers, 8, 128, 1024], f32r)   # [li][di][p][head-major col]
    wk_in = par("wk", [n_layers, 8, 128, 1024], f32r)
    wo_in = par("wo", [n_layers, 8, 128, 1024], f32r)   # [li][hp][p][e]
    w1_in = par("w1", [n_layers, 4, 8, 128, 8, 128], f32r)  # [li][jg][di][p][jj][q]
    w2_in = par("w2", [n_layers, 32, 128, 1024], f32r)  # [li][jc][p][e]
    b1c_in = par("b1c", [n_layers, 32, 128], f32)
    b2c_in = par("b2c", [n_layers, 8, 128], f32)
    g1r_in = par("g1r", [n_layers, 8, 128], f32r)
    b1r_in = par("b1r", [n_layers, 8, 128], f32r)
    g2r_in = par("g2r", [n_layers, 8, 128], f32r)
    b2r_in = par("b2r", [n_layers, 8, 128], f32r)
    km_in = par("km", [8, 128], f32r)                   # 0/1 keep-mask, all 1024 keys
    kmo_in = par("kmo", [2, 128], f32r)                 # keep-mask for own 256 keys
    ones_in = par("ones", [128, 256], f32r)
    out_par = par("out", [LC, D], f32, out=True)
    DBG = os.environ.get("KERNEL_DEBUG", "0") == "1"
    if DBG:
        dbg_q = par("dbg_q", [128, 8, 256], f32, out=True)
        dbg_kt = par("dbg_kt", [128, 8, 1024], f32, out=True)
        dbg_kn = par("dbg_kn", [128, 8, 1024], f32, out=True)
        dbg_w0 = par("dbg_w0", [128, 8, 256], f32, out=True)
        dbg_at = par("dbg_at", [128, 8, 256], f32, out=True)
        dbg_r1 = par("dbg_r1", [128, 8, 256], f32, out=True)
        dbg_x1 = par("dbg_x1", [128, 8, 256], f32, out=True)

    EPS = 1e-5
    INV_D = 1.0 / float(D)
    INV_SQ = 0.125  # 1/sqrt(HD)

    with tile.TileContext(nc) as tc:
        ctx = contextlib.ExitStack()
        with ctx:
            sbc = ctx.enter_context(tc.tile_pool(name="const", bufs=1))
            sbx = ctx.enter_context(tc.tile_pool(name="xt", bufs=2))
            sbk = ctx.enter_context(tc.tile_pool(name="kbuf", bufs=1))
            sbq = ctx.enter_context(tc.tile_pool(name="qbuf", bufs=1))
            sbw = ctx.enter_context(tc.tile_pool(name="wts", bufs=4))
            sba = ctx.enter_context(tc.tile_pool(name="act", bufs=2))
            sbh = ctx.enter_context(tc.tile_pool(name="hbuf", bufs=1))
            sbs = ctx.enter_context(tc.tile_pool(name="small", bufs=4))
            psp = ctx.enter_context(tc.tile_pool(name="ps", bufs=2, space="PSUM"))
            drp = ctx.enter_context(tc.tile_pool(name="dram", bufs=2, space="DRAM"))

            ones = sbc.tile([128, 256], f32r, name="ones_t")
            nc.sync.dma_start(out=ones[:, :], in_=ones_in[:, :])
            km = sbc.tile([128, 8], f32r, name="km_t")
            nc.sync.dma_start(out=km[:, :], in_=km_in.rearrange("m p -> p m"))
            kmo = sbc.tile([128, 2], f32r, name="kmo_t")
            nc.sync.dma_start(out=kmo[:, :], in_=kmo_in.rearrange("m p -> p m"))
            ident = sbc.tile([128, 128], f32, name="ident_t")
            make_identity(nc, ident[:, :])

            xT = sbx.tile([128, 8, LC], f32r, tag="xT", name="xT0")
            nc.sync.dma_start(out=xT[:, :, :], in_=xT_in.rearrange("e p l -> p e l"))

            def layer_norm(resid, xT_out, gr_dram, br_dram, uid):
                """xT_out = LN(resid) * g + b, all per-column-l stats."""
                grt = sbs.tile([1, 8, 128], f32r, tag="gr", bufs=1, name=f"gr_{uid}")
                nc.sync.dma_start(out=grt[:, :, :], in_=gr_dram.unsqueeze(0))
                brt = sbs.tile([1, 8, 128], f32r, tag="br", bufs=1, name=f"br_{uid}")
                nc.sync.dma_start(out=brt[:, :, :], in_=br_dram.unsqueeze(0))

                ps_st = psp.tile([128, 2048], f32, tag="ps", name=f"psst_{uid}")
                for ei in range(8):
                    st, sp = ei == 0, ei == 7
                    sq1 = sbs.tile([128, 256], f32r, tag="sqtmp", bufs=2,
                                   name=f"sq_{uid}_{ei}")
                    nc.scalar.activation(sq1[:, :], resid[:, ei, :].bitcast(f32),
                                         AF.Square)
                    nc.tensor.matmul(ps_st[0:1, 0:256], ones[:, 0:1],
                                     resid[:, ei, :], start=st, stop=sp)
                    nc.tensor.matmul(ps_st[0:1, 256:512], ones[:, 0:1],
                                     sq1[:, :], start=False, stop=sp)
                mu = sbs.tile([1, 256], f32, tag="st1", bufs=1, name=f"mu_{uid}")
                nc.vector.tensor_scalar_mul(mu[:, :], ps_st[0:1, 0:256], INV_D)
                ex2 = sbs.tile([1, 256], f32, tag="st2", bufs=1, name=f"ex2_{uid}")
                nc.vector.tensor_scalar_mul(ex2[:, :], ps_st[0:1, 256:512], INV_D)
                mu2 = sbs.tile([1, 256], f32, tag="st3", bufs=1, name=f"mu2_{uid}")
                nc.vector.tensor_mul(mu2[:, :], mu[:, :], mu[:, :])
                var = sbs.tile([1, 256], f32, tag="st4", bufs=1, name=f"var_{uid}")
                nc.vector.scalar_tensor_tensor(
                    out=var[:, :], in0=ex2[:, :], scalar=EPS, in1=mu2[:, :],
                    op0=ALU.add, op1=ALU.subtract)
                sd = sbs.tile([1, 256], f32, tag="st5", bufs=1, name=f"sd_{uid}")
                nc.scalar.activation(sd[:, :], var[:, :], AF.Sqrt)
                rstd = sbs.tile([1, 256], f32r, tag="st6", bufs=1, name=f"rstd_{uid}")
                with nc.allow_low_precision(reason="f32r rounding ok"):
                    nc.vector.reciprocal(rstd[:, :], sd[:, :])
                nmr = sbs.tile([1, 256], f32r, tag="st7", bufs=1, name=f"nmr_{uid}")
                nc.vector.scalar_tensor_tensor(
                    out=nmr[:, :], in0=mu[:, :], scalar=-1.0, in1=rstd[:, :].bitcast(f32),
                    op0=ALU.mult, op1=ALU.mult)
                ps_rg = psp.tile([128, 2048], f32, tag="ps", name=f"psrg_{uid}")
                ps_nb = psp.tile([128, 2048], f32, tag="ps", name=f"psnb_{uid}")
                for ei in range(8):
                    nc.tensor.matmul(ps_rg[:, ei * 256:(ei + 1) * 256],
                                     grt[0:1, ei, :], rstd[:, :],
                                     start=True, stop=True)
                    nc.tensor.matmul(ps_nb[:, ei * 256:(ei + 1) * 256],
                                     grt[0:1, ei, :], nmr[:, :],
                                     start=True, stop=False)
                    nc.tensor.matmul(ps_nb[:, ei * 256:(ei + 1) * 256],
                                     brt[0:1, ei, :], ones[0:1, 0:256],
                                     start=False, stop=True)
                for ei in range(8):
                    tmp = sbs.tile([128, 256], f32, tag="lntmp", name=f"lnt_{uid}_{ei}", bufs=2)
                    nc.vector.tensor_mul(tmp[:, :], resid[:, ei, :].bitcast(f32),
                                         ps_rg[:, ei * 256:(ei + 1) * 256])
                    nc.vector.tensor_tensor(
                        out=xT_out[:, ei, :], in0=tmp[:, :],
                        in1=ps_nb[:, ei * 256:(ei + 1) * 256], op=ALU.add)

            for rep in range(reps):
              for li in range(n_layers):
                  # ---------------- K phase: own-chunk kT and k_nat ----------
                  mark(f"L{li}.k")
                  ps_kt = psp.tile([128, 2048], f32, tag="ps", name=f"pskt_{li}")
                  ps_kn = psp.tile([128, 2048], f32, tag="ps", name=f"pskn_{li}")
                  for di in range(8):
                      wkt = sbw.tile([128, 1024], f32r, tag="w", name=f"wk_{li}_{di}")
                      nc.sync.dma_start(out=wkt[:, :], in_=wk_in[li, di])
                      st, sp = di == 0, di == 7
                      for hp in range(8):
                          nc.tensor.matmul(
                              ps_kt[:, hp * 256:(hp + 1) * 256],
                              wkt[:, hp * 128:(hp + 1) * 128],
                              xT[:, di, :], start=st and hp % 2 == 0, stop=sp)
                      for ms in range(2):
                          for nn_ in range(2):
                              nc.tensor.matmul(
                                  ps_kn[:, ms * 1024 + nn_ * 512:ms * 1024 + (nn_ + 1) * 512],
                                  xT[:, di, ms * 128:(ms + 1) * 128],
                                  wkt[:, nn_ * 512:(nn_ + 1) * 512], start=st, stop=sp)
                  kTc = sbs.tile([128, 2048], f32r, tag="kstage", bufs=1,
                                 name=f"kTc_{li}")
                  nc.vector.tensor_copy(kTc[:, :], ps_kt[:, :])
                  knc = sbs.tile([128, 2048], f32r, tag="kstage", bufs=1,
                                 name=f"knc_{li}")
                  for ms in range(2):
                      # zero masked keys (keys are partitions here) of own chunk
                      nc.vector.tensor_scalar_mul(
                          knc[:, ms * 1024:(ms + 1) * 1024],
                          ps_kn[:, ms * 1024:(ms + 1) * 1024],
                          kmo[:, ms:ms + 1].bitcast(f32))

                  # one merged AllGather per layer: blocks 0-7 = kT chunk,
                  # blocks 8-15 = k_nat chunk
                  agi = drp.tile([16, 128, 256], f32r, tag="agi", name=f"agi_{li}")
                  ago = drp.tile([4, 16, 128, 256], f32r, tag="ago", name=f"ago_{li}")
                  nc.sync.dma_start(out=agi[0:8].rearrange("h p m -> p h m"),
                                    in_=kTc[:, :].rearrange("p (h m) -> p h m", h=8))
                  nc.sync.dma_start(out=agi[8:16].rearrange("b p c -> p b c"),
                                    in_=knc[:, :].rearrange("p (b c) -> p b c", b=8))
                  if FAKE_AG:
                      for r in range(4):
                          nc.sync.dma_start(out=ago[r], in_=agi[:, :, :])
                  else:
                      nc.gpsimd.collective_compute(
                          "AllGather", mybir.AluOpType.bypass,
                          replica_groups=[[0, 1, 2, 3], [4, 5, 6, 7]],
                          ins=[agi.opt()], outs=[ago.opt()])

                  mark(f"L{li}.q")
                  # ---------------- Q phase (overlaps AG) --------------------
                  ps_q = psp.tile([128, 2048], f32, tag="ps", name=f"psq_{li}")
                  for di in range(8):
                      wqt = sbw.tile([128, 1024], f32r, tag="w", name=f"wq_{li}_{di}")
                      nc.sync.dma_start(out=wqt[:, :], in_=wq_in[li, di])
                      for hp in range(8):
                          nc.tensor.matmul(
                              ps_q[:, hp * 256:(hp + 1) * 256],
                              wqt[:, hp * 128:(hp + 1) * 128],
                              xT[:, di, :], start=di == 0 and hp % 2 == 0,
                              stop=di == 7)
                  qT = sbq.tile([128, 8, 256], f32r, tag="qT", name=f"qT_{li}")
                  nc.vector.tensor_copy(qT[:, :, :],
                                        ps_q[:, :].rearrange("p (h m) -> p h m", h=8))

                  kT = sbk.tile([128, 8, 1024], f32r, tag="kT", name=f"kT_{li}")
                  kn = sbk.tile([128, 8, 1024], f32r, tag="kn", name=f"kn_{li}")
                  for r in range(4):
                      nc.sync.dma_start(
                          out=kT[:, :, r * 256:(r + 1) * 256],
                          in_=ago[r, 0:8].rearrange("h p m -> p h m"))
                      nc.sync.dma_start(
                          out=kn[:, r * 2:(r + 1) * 2, :].rearrange("p s r2 -> p (s r2)").rearrange("p (b c) -> p b c", b=8),
                          in_=ago[r, 8:16].rearrange("b p c -> p b c"))
                  if DBG and li == 0:
                      nc.sync.dma_start(out=dbg_q.rearrange("p h m -> p h m"),
                                        in_=qT[:, :, :].bitcast(f32))
                      nc.sync.dma_start(out=dbg_kt[:, :, :], in_=kT[:, :, :].bitcast(f32))
                      nc.sync.dma_start(out=dbg_kn[:, :, :], in_=kn[:, :, :].bitcast(f32))

                  mark(f"L{li}.attn")
                  # ---------------- attention, head by head ------------------
                  attnT = sba.tile([128, 8, 256], f32r, tag="attnT", bufs=1, name=f"attnT_{li}")
                  for h in range(H):
                      hp, sub = h // 2, h % 2
                      lo, hi = sub * 64, sub * 64 + 64
                      ps_s = psp.tile([128, 2048], f32, tag="ps", name=f"pss_{li}_{h}")
                      for mi in range(8):
                          nc.tensor.matmul(
                              ps_s[:, mi * 256:(mi + 1) * 256],
                              kT[lo:hi, hp, mi * 128:(mi + 1) * 128],
                              qT[lo:hi, hp, :], start=True, stop=True)
                      wT = sba.tile([128, 8, 256], f32r, tag="wT", name=f"wT_{li}_{h}")
                      for w_ in range(2):
                          nc.scalar.activation(
                              wT[:, w_ * 4:(w_ + 1) * 4, :],
                              ps_s[:, w_ * 1024:(w_ + 1) * 1024].rearrange(
                                  "p (i m) -> p i m", i=4),
                              AF.Exp, bias=0.0, scale=INV_SQ)
                      if DBG and li == 0 and h == 0:
                          nc.sync.dma_start(out=dbg_w0[:, :, :], in_=wT[:, :, :].bitcast(f32))
                      ps_o = psp.tile([128, 2048], f32, tag="ps", name=f"pso_{li}_{h}")
                      for mi in range(8):
                          st, sp = mi == 0, mi == 7
                          nc.tensor.matmul(
                              ps_o[0:64, 0:256],
                              kn[:, mi, h * 64:(h + 1) * 64],
                              wT[:, mi, :], start=st, stop=sp)
                          nc.tensor.matmul(
                              ps_o[0:1, 256:512],
                              km[:, mi:mi + 1],
                              wT[:, mi, :], start=False, stop=sp)
                      rcp = sbs.tile([1, 256], f32r, tag="rcp", bufs=2, name=f"rcp_{li}_{h}")
                      with nc.allow_low_precision(reason="f32r rounding ok"):
                          nc.vector.reciprocal(rcp[:, :], ps_o[0:1, 256:512])
                      nc.tensor.matmul(
                          ps_o[0:64, 512:768], ones[0:1, 0:64], rcp[:, :],
                          start=True, stop=True)
                      rep = sbs.tile([128, 256], f32, tag="rep", name=f"rep_{li}_{h}", bufs=2)
                      nc.vector.tensor_copy(rep[0:64, :], ps_o[0:64, 512:768])
                      nc.vector.tensor_mul(attnT[lo:hi, hp, :],
                                           ps_o[0:64, 0:256],
                                           rep[0:64, :])

                  mark(f"L{li}.wo")
                  # ---------------- Wo + residual + LN1 ----------------------
                  ps_y = psp.tile([128, 2048], f32, tag="ps", name=f"psy_{li}")
                  for hp in range(8):
                      wot = sbw.tile([128, 1024], f32r, tag="w", name=f"wo_{li}_{hp}")
                      nc.sync.dma_start(out=wot[:, :], in_=wo_in[li, hp])
                      for ei in range(8):
                          nc.tensor.matmul(
                              ps_y[:, ei * 256:(ei + 1) * 256],
                              wot[:, ei * 128:(ei + 1) * 128],
                              attnT[:, hp, :], start=hp == 0 and ei % 2 == 0,
                              stop=hp == 7)
                  if DBG and li == 0:
                      nc.sync.dma_start(out=dbg_at[:, :, :], in_=attnT[:, :, :].bitcast(f32))
                  resid = sba.tile([128, 8, 256], f32r, tag="resid", bufs=1, name=f"res1_{li}")
                  for ei in range(8):
                      nc.vector.scalar_tensor_tensor(
                          out=resid[:, ei, :], in0=ps_y[:, ei * 256:(ei + 1) * 256],
                          scalar=0.0, in1=xT[:, ei, :].bitcast(f32),
                          op0=ALU.add, op1=ALU.add)

                  if DBG and li == 0:
                      nc.sync.dma_start(out=dbg_r1[:, :, :], in_=resid[:, :, :].bitcast(f32))
                  xT = sbx.tile([128, 8, LC], f32r, tag="xT", name=f"xT_{li}a")
                  layer_norm(resid, xT, g1r_in[li], b1r_in[li], f"{li}a")
                  if DBG and li == 0:
                      nc.sync.dma_start(out=dbg_x1[:, :, :], in_=xT[:, :, :].bitcast(f32))

                  mark(f"L{li}.ffn1")
                  # ---------------- FFN --------------------------------------
                  hT = sbh.tile([128, 32, 256], f32r, tag="hT", name=f"hT_{li}")
                  b1c = sbs.tile([128, 32], f32, tag="b1c", name=f"b1c_{li}")
                  nc.sync.dma_start(out=b1c[:, :], in_=b1c_in[li].rearrange("j p -> p j"))
                  for jg in range(4):
                      ps_h = psp.tile([128, 2048], f32, tag="ps", name=f"psh_{li}_{jg}")
                      for di in range(8):
                          w1t = sbw.tile([128, 8, 128], f32r, tag="w",
                                         name=f"w1_{li}_{jg}_{di}")
                          nc.sync.dma_start(out=w1t[:, :, :], in_=w1_in[li, jg, di])
                          for jj in range(8):
                              nc.tensor.matmul(
                                  ps_h[:, jj * 256:(jj + 1) * 256],
                                  w1t[:, jj, :],
                                  xT[:, di, :], start=di == 0 and jj % 2 == 0,
                                  stop=di == 7)
                      for jj in range(8):
                          nc.scalar.activation(
                              hT[:, jg * 8 + jj, :], ps_h[:, jj * 256:(jj + 1) * 256],
                              AF.Relu, bias=b1c[:, jg * 8 + jj:jg * 8 + jj + 1], scale=1.0)

                  mark(f"L{li}.ffn2")
                  ps_f = psp.tile([128, 2048], f32, tag="ps", name=f"psf_{li}")
                  for jc in range(32):
                      w2t = sbw.tile([128, 1024], f32r, tag="w", name=f"w2_{li}_{jc}")
                      nc.sync.dma_start(out=w2t[:, :], in_=w2_in[li, jc])
                      for ei in range(8):
                          nc.tensor.matmul(
                              ps_f[:, ei * 256:(ei + 1) * 256],
                              w2t[:, ei * 128:(ei + 1) * 128],
                              hT[:, jc, :], start=jc == 0 and ei % 2 == 0,
                              stop=jc == 31)
                  b2c = sbs.tile([128, 8], f32, tag="b2c", name=f"b2c_{li}")
                  nc.sync.dma_start(out=b2c[:, :], in_=b2c_in[li].rearrange("e p -> p e"))
                  resid2 = sba.tile([128, 8, 256], f32r, tag="resid", bufs=1, name=f"res2_{li}")
                  for ei in range(8):
                      nc.vector.scalar_tensor_tensor(
                          out=resid2[:, ei, :], in0=ps_f[:, ei * 256:(ei + 1) * 256],
                          scalar=b2c[:, ei:ei + 1], in1=xT[:, ei, :].bitcast(f32),
                          op0=ALU.add, op1=ALU.add)

                  mark(f"L{li}.ln2")
                  xT = sbx.tile([128, 8, LC], f32r, tag="xT", name=f"xT_{li}b")
                  layer_norm(resid2, xT, g2r_in[li], b2r_in[li], f"{li}b")

            mark("out")
            # ---------------- output: transpose back --------------------
            for lj in range(2):
                ps_t = psp.tile([128, 2048], f32, tag="ps", name=f"pst_{lj}")
                for ei in range(8):
                    nc.tensor.transpose(
                        ps_t[:, ei * 256:ei * 256 + 128],
                        xT[:, ei, lj * 128:(lj + 1) * 128].bitcast(f32),
                        ident[:, :])
                outp = sbs.tile([128, 1024], f32, tag="outp", bufs=1,
                                name=f"outp_{lj}")
                nc.vector.tensor_copy(
                    outp[:, :].rearrange("p (e m) -> p e m", e=8),
                    ps_t[:, :].rearrange("p (e m) -> p e m", e=8)[:, :, 0:128])
                nc.sync.dma_start(out=out_par[lj * 128:(lj + 1) * 128, :],
                                  in_=outp[:, :])
    nc.finalize()
    return nc


def _host_prep(inputs, n_layers):
    """Host-side preprocessing: embedding+PE, weight reshapes, per-core maps."""
    tokens = np.asarray(inputs["tokens"])
    mask = np.asarray(inputs["self_attn_mask"])
    emb = np.asarray(inputs["emb"], dtype=np.float32)
    Wq = np.asarray(inputs["Wq"], dtype=np.float32)
    Wk = np.asarray(inputs["Wk"], dtype=np.float32)
    Wo = np.asarray(inputs["Wo"], dtype=np.float32)
    W1 = np.asarray(inputs["W1"], dtype=np.float32)
    b1 = np.asarray(inputs["b1"], dtype=np.float32)
    W2 = np.asarray(inputs["W2"], dtype=np.float32)
    b2 = np.asarray(inputs["b2"], dtype=np.float32)
    g1 = np.asarray(inputs["ln1_g"], dtype=np.float32)
    be1 = np.asarray(inputs["ln1_b"], dtype=np.float32)
    g2 = np.asarray(inputs["ln2_g"], dtype=np.float32)
    be2 = np.asarray(inputs["ln2_b"], dtype=np.float32)

    # input block (exact f32, same ops as reference)
    emb0 = emb.copy()
    emb0[PAD] = 0.0
    x = emb0[tokens] * np.float32(D ** 0.5)
    pos = np.arange(L, dtype=np.float32)[:, None]
    i = np.arange(D // 2, dtype=np.float32)[None, :]
    angle = pos / (10000.0 ** (2.0 * i / D))
    pe = np.zeros((L, D), np.float32)
    pe[:, 0::2] = np.sin(angle)
    pe[:, 1::2] = np.cos(angle)
    x = (x + pe[None]).astype(np.float32)  # [B, L, D]

    # head-major permutation: new col r = h*64+d'  <- old col d'*H + h
    r = np.arange(D)
    perm = (r % HD) * H + (r // HD)
    Wq_p = np.ascontiguousarray(Wq[:n_layers][:, :, perm])
    Wk_p = np.ascontiguousarray(Wk[:n_layers][:, :, perm])
    Wo_p = np.ascontiguousarray(Wo[:n_layers][:, perm, :])

    wq_d = Wq_p.reshape(n_layers, 8, 128, 1024)
    wk_d = Wk_p.reshape(n_layers, 8, 128, 1024)
    wo_d = Wo_p.reshape(n_layers, 8, 128, 1024)
    w1_d = np.ascontiguousarray(
        W1[:n_layers].reshape(n_layers, 8, 128, 4, 8, 128).transpose(0, 3, 1, 2, 4, 5))
    w2_d = np.ascontiguousarray(W2[:n_layers].reshape(n_layers, 32, 128, 1024))
    b1c = b1[:n_layers].reshape(n_layers, 32, 128)
    b2c = b2[:n_layers].reshape(n_layers, 8, 128)
    g1r = g1[:n_layers].reshape(n_layers, 8, 128)
    b1r = be1[:n_layers].reshape(n_layers, 8, 128)
    g2r = g2[:n_layers].reshape(n_layers, 8, 128)
    b2r = be2[:n_layers].reshape(n_layers, 8, 128)
    ones = np.ones((128, 256), np.float32)

    shared = dict(wq=wq_d, wk=wk_d, wo=wo_d, w1=w1_d, w2=w2_d,
                  b1c=b1c, b2c=b2c, g1r=g1r, b1r=b1r, g2r=g2r, b2r=b2r,
                  ones=ones)

    in_maps = []
    for c in range(NCORES):
        b = c // 4
        j = c % 4
        xT0 = np.ascontiguousarray(
            x[b, j * LC:(j + 1) * LC, :].T).reshape(8, 128, LC)
        keep = (~mask[b, 0, :, 0]).astype(np.float32)  # 1.0 where key kept
        km = keep.reshape(8, 128)
        kmo = keep[j * LC:(j + 1) * LC].reshape(2, 128)
        m = dict(shared)
        m.update(xT_in=xT0, km=km, kmo=kmo)
        in_maps.append(m)
    return x, in_maps


def kernel(**inputs) -> np.ndarray:
    from concourse.bass_utils import run_bass_kernel_spmd

    n_layers = _DEV_NL
    if "nc" not in _cache or _cache.get("nl") != n_layers:
        _cache["nc"] = _build_nc(n_layers)
        _cache["nl"] = n_layers
    nc = _cache["nc"]

    _, in_maps = _host_prep(inputs, n_layers)
    res = run_bass_kernel_spmd(nc, in_maps, core_ids=list(range(NCORES)))
    out = np.empty((B, L, D), np.float32)
    for c in range(NCORES):
        b, j = c // 4, c % 4
        out[b, j * LC:(j + 1) * LC, :] = res.results[c]["out"]
    return out



# revision 45
# speedup vs baseline: 1.7668x; 1.7668x over previous
"""Trainium2 Bass kernel for nn_Encoder (4-layer dense transformer encoder).

Sharding: sequence-sharded data parallel. 8 cores = 2 batches x 4 sequence
chunks of 256 tokens. Per layer each core computes its own K chunk (both the
[HD, keys] and [keys, HD] layouts) and AllGathers K within its batch's
4-core group. Activations stay transposed (xT [D, L_local]) so every matmul
contracts over the partition dim.

Precision: weights and the AllGathered K payload are bf16 (PE runs bf16 at
full rate); activations and LN statistics stay fp32/fp32r. The attention
softmax denominator rides as a 65th stationary column (the 0/1 keep-mask,
interleaved into the K payload per head), so one matmul chain produces both
the numerator and denominator.

Self-contained: hardcodes all shapes; host side does the embedding gather,
positional encoding, weight permutations/reshapes, and output assembly.
"""
import os
import numpy as np

B, L, D, H, M, NL, V = 2, 1024, 1024, 16, 4096, 4, 32000
HD = D // H          # 64
LC = 256             # local sequence chunk per core
NCORES = 8
PAD = 0

_DEV_NL = int(os.environ.get("KERNEL_NL", str(NL)))

_cache = {}
PHASE_LOG = []

KCOLS = 2048 + 2             # AG payload cols per partition: kT | own-chunk mask


def _build_nc(n_layers, reps=1):
    import os
    import contextlib
    import concourse.mybir as mybir
    import concourse.tile as tile
    from concourse import bacc
    from concourse.masks import make_identity

    f32 = mybir.dt.float32
    f32r = mybir.dt.float32r
    bf16 = mybir.dt.bfloat16
    AF = mybir.ActivationFunctionType
    ALU = mybir.AluOpType

    FAKE_AG = os.environ.get("KERNEL_FAKE_AG", "0") == "1"
    MULTIQ = os.environ.get("KERNEL_MULTIQ", "1") == "1"
    TRUNC = os.environ.get("KERNEL_TRUNC", "")
    nc = bacc.Bacc(None, target_bir_lowering=False, num_devices=NCORES)
    PHASE_LOG.clear()

    def mark(label):
        PHASE_LOG.append((label, len(nc.inst_map)))

    def par(name, shape, dt, out=False):
        return nc.declare_dram_parameter(name, list(shape), dt, isOutput=out)

    xT_in = par("xT_in", [8, 128, LC], f32r)
    wq_in = par("wq", [n_layers, 8, 128, 1024], bf16)   # [li][di][p][head-major col]
    wk_in = par("wk", [n_layers, 8, 128, 1024], bf16)
    wo_in = par("wo", [n_layers, 8, 128, 1024], bf16)   # [li][hp][p][e]
    w1_in = par("w1", [n_layers, 4, 8, 128, 1024], bf16)  # [li][jg][di][p][jj*128+q]
    w2_in = par("w2", [n_layers, 4, 8, 128, 1024], bf16)  # [li][jg][jc][p][e]
    b1c_in = par("b1c", [n_layers, 128, 32], f32)       # host-transposed
    b2c_in = par("b2c", [n_layers, 128, 8], f32)
    lnp_in = par("lnp", [n_layers, 128, 4, 8], f32)     # [p][g1,b1,g2,b2][ei]
    kmo_in = par("kmo", [128, 2], f32r)                 # keep-mask for own 256 keys
    ones_in = par("ones", [128, 256], f32r)
    out_par = par("out", [LC, D], f32, out=True)

    EPS = 1e-5
    INV_D = 1.0 / float(D)
    INV_SQ = 0.125  # 1/sqrt(HD)

    agq = nc.gpsimd if MULTIQ else nc.sync    # AG-chain DMA queue
    smq = nc.scalar if MULTIQ else nc.sync    # small-param DMA queue

    with tile.TileContext(nc) as tc:
        ctx = contextlib.ExitStack()
        with ctx:
            sbc = ctx.enter_context(tc.tile_pool(name="const", bufs=1))
            sbx = ctx.enter_context(tc.tile_pool(name="xt", bufs=2))
            sbk = ctx.enter_context(tc.tile_pool(name="kbuf", bufs=1))
            sbq = ctx.enter_context(tc.tile_pool(name="qbuf", bufs=1))
            sbw = ctx.enter_context(tc.tile_pool(name="wts", bufs=6))
            sba = ctx.enter_context(tc.tile_pool(name="act", bufs=2))
            sbh = ctx.enter_context(tc.tile_pool(name="hbuf", bufs=1))
            sbs = ctx.enter_context(tc.tile_pool(name="small", bufs=4))
            psp = ctx.enter_context(tc.tile_pool(name="ps", bufs=4, space="PSUM"))
            drp = ctx.enter_context(tc.tile_pool(name="dram", bufs=2, space="DRAM"))

            def pst(name):
                return psp.tile([128, 1024], f32, tag="ps", name=name)

            ones = sbc.tile([128, 256], f32r, name="ones_t")
            nc.sync.dma_start(out=ones[:, :], in_=ones_in[:, :])
            kmo = sbc.tile([128, 2], f32r, name="kmo_t")
            nc.sync.dma_start(out=kmo[:, :], in_=kmo_in[:, :])
            ident = sbc.tile([128, 128], f32, name="ident_t")
            make_identity(nc, ident[:, :])
            identb = sbc.tile([128, 128], bf16, name="identb_t")
            make_identity(nc, identb[:, :])

            xT = sbx.tile([128, 8, LC], f32r, tag="xT", name="xT0")
            nc.sync.dma_start(out=xT[:, :, :], in_=xT_in.rearrange("e p l -> p e l"))
            # bf16 shadow of xT for matmul inputs (PE disallows f32r x bf16)
            xTb = sbx.tile([128, 8, LC], bf16, tag="xTb", name="xTb0")
            nc.scalar.activation(xTb[:, 0:2, :], xT[:, 0:2, :].bitcast(f32), AF.Copy)
            nc.scalar.activation(xTb[:, 2:8, :], xT[:, 2:8, :].bitcast(f32), AF.Copy)

            def layer_norm(resid, xT_out, xTb_out, lnt2, pi, uid):
                """xT_out = LN(resid) * g + b, per-column-l stats.
                lnt2 is [128, 4, 8]: g at [:, 2*pi, ei], b at [:, 2*pi+1, ei]."""
                ps_st = pst(f"psst_{uid}")
                for ei in range(8):
                    st, sp = ei == 0, ei == 7
                    sq1 = sbs.tile([128, 256], f32r, tag="sqtmp", bufs=2,
                                   name=f"sq_{uid}_{ei}")
                    nc.scalar.activation(sq1[:, :], resid[:, ei, :].bitcast(f32),
                                         AF.Square)
                    nc.tensor.matmul(ps_st[0:1, 0:256], ones[:, 0:1],
                                     resid[:, ei, :], start=st, stop=False)
                    nc.tensor.matmul(ps_st[0:1, 256:512], ones[:, 0:1],
                                     sq1[:, :], start=False, stop=sp)
                mu = sbs.tile([1, 256], f32, tag="st1", bufs=1, name=f"mu_{uid}")
                nc.vector.tensor_scalar_mul(mu[:, :], ps_st[0:1, 0:256], INV_D)
                ex2 = sbs.tile([1, 256], f32, tag="st2", bufs=1, name=f"ex2_{uid}")
                nc.vector.tensor_scalar_mul(ex2[:, :], ps_st[0:1, 256:512], INV_D)
                mu2 = sbs.tile([1, 256], f32, tag="st3", bufs=1, name=f"mu2_{uid}")
                nc.vector.tensor_mul(mu2[:, :], mu[:, :], mu[:, :])
                var = sbs.tile([1, 256], f32, tag="st4", bufs=1, name=f"var_{uid}")
                nc.vector.scalar_tensor_tensor(
                    out=var[:, :], in0=ex2[:, :], scalar=EPS, in1=mu2[:, :],
                    op0=ALU.add, op1=ALU.subtract)
                sd = sbs.tile([1, 256], f32, tag="st5", bufs=1, name=f"sd_{uid}")
                nc.scalar.activation(sd[:, :], var[:, :], AF.Sqrt)
                rstd = sbs.tile([1, 256], f32r, tag="st6", bufs=1, name=f"rstd_{uid}")
                with nc.allow_low_precision(reason="f32r rounding ok"):
                    nc.vector.reciprocal(rstd[:, :], sd[:, :])
                nmr = sbs.tile([1, 256], f32r, tag="st7", bufs=1, name=f"nmr_{uid}")
                nc.vector.scalar_tensor_tensor(
                    out=nmr[:, :], in0=mu[:, :], scalar=-1.0, in1=rstd[:, :].bitcast(f32),
                    op0=ALU.mult, op1=ALU.mult)
                # broadcast rstd / (-mu*rstd) along partitions via one matmul each
                ps_bc = pst(f"psbc_{uid}")
                nc.tensor.matmul(ps_bc[:, 0:256], ones[0:1, 0:128], rstd[:, :],
                                 start=True, stop=False)
                nc.tensor.matmul(ps_bc[:, 256:512], ones[0:1, 0:128], nmr[:, :],
                                 start=False, stop=True)
                for ei in range(8):
                    tmp = sbs.tile([128, 256], f32, tag="lntmp", name=f"lnt_{uid}_{ei}", bufs=2)
                    nc.vector.tensor_mul(tmp[:, :], resid[:, ei, :].bitcast(f32),
                                         ps_bc[:, 0:256])
                    tmp2 = sbs.tile([128, 256], f32, tag="lntmp2", name=f"lnu_{uid}_{ei}", bufs=2)
                    nc.vector.tensor_tensor(
                        out=tmp2[:, :], in0=tmp[:, :], in1=ps_bc[:, 256:512],
                        op=ALU.add)
                    nc.vector.tensor_scalar(
                        out=xT_out[:, ei, :], in0=tmp2[:, :],
                        scalar1=lnt2[:, 2 * pi, ei:ei + 1],
                        scalar2=lnt2[:, 2 * pi + 1, ei:ei + 1],
                        op0=ALU.mult, op1=ALU.add)
                    nc.scalar.activation(xTb_out[:, ei, :],
                                         xT_out[:, ei, :].bitcast(f32), AF.Copy)

            for rep in range(reps):
              for li in range(n_layers):
                  lnt2 = sbs.tile([128, 4, 8], f32, tag="lnt", bufs=2,
                                  name=f"lnt_{li}")
                  smq.dma_start(out=lnt2[:, :, :], in_=lnp_in[li])

                  # ---------------- K phase: own-chunk kT --------------------
                  mark(f"L{li}.k")
                  ps_kt = [pst(f"pskt_{li}_{t}") for t in range(2)]
                  for dh in range(2):
                      wkt = sbw.tile([128, 4, 1024], bf16, tag="w",
                                     name=f"wk_{li}_{dh}")
                      nc.sync.dma_start(
                          out=wkt[:, :, :],
                          in_=wk_in[li, dh * 4:(dh + 1) * 4].rearrange("d p e -> p d e"))
                      for dj in range(4):
                          di = dh * 4 + dj
                          st, sp = di == 0, di == 7
                          for hp in range(8):
                              nc.tensor.matmul(
                                  ps_kt[hp // 4][:, (hp % 4) * 256:(hp % 4) * 256 + 256],
                                  wkt[:, dj, hp * 128:(hp + 1) * 128],
                                  xTb[:, di, :], start=st and hp % 2 == 0,
                                  stop=sp and hp % 2 == 1)
                  # payload: kT cols + own-chunk keep-mask (2 cols)
                  kTc = sbs.tile([128, KCOLS], bf16, tag="ktstage", bufs=1,
                                 name=f"kTc_{li}")
                  for t in range(2):
                      nc.vector.tensor_copy(kTc[:, t * 1024:(t + 1) * 1024],
                                            ps_kt[t][:, :])
                  nc.vector.tensor_copy(kTc[:, 2048:KCOLS],
                                        kmo[:, :].bitcast(f32))

                  # one AllGather per layer
                  agi = drp.tile([1, 128, KCOLS], bf16, tag="agi", name=f"agi_{li}")
                  ago = drp.tile([4, 1, 128, KCOLS], bf16, tag="ago", name=f"ago_{li}")
                  agq.dma_start(out=agi[0, :, 0:1024], in_=kTc[:, 0:1024])
                  agq.dma_start(out=agi[0, :, 1024:KCOLS], in_=kTc[:, 1024:KCOLS])
                  if FAKE_AG:
                      for r in range(4):
                          agq.dma_start(out=ago[r], in_=agi[:, :, :])
                  else:
                      nc.gpsimd.collective_compute(
                          "AllGather", mybir.AluOpType.bypass,
                          replica_groups=[[0, 1, 2, 3], [4, 5, 6, 7]],
                          ins=[agi.opt()], outs=[ago.opt()])

                  if TRUNC == "k0":
                      continue
                  mark(f"L{li}.q")
                  # ---------------- Q phase (overlaps AG) --------------------
                  ps_q = [pst(f"psq_{li}_{t}") for t in range(2)]
                  for dh in range(2):
                      wqt = sbw.tile([128, 4, 1024], bf16, tag="w",
                                     name=f"wq_{li}_{dh}")
                      nc.sync.dma_start(
                          out=wqt[:, :, :],
                          in_=wq_in[li, dh * 4:(dh + 1) * 4].rearrange("d p e -> p d e"))
                      for dj in range(4):
                          di = dh * 4 + dj
                          for hp in range(8):
                              nc.tensor.matmul(
                                  ps_q[hp // 4][:, (hp % 4) * 256:(hp % 4) * 256 + 256],
                                  wqt[:, dj, hp * 128:(hp + 1) * 128],
                                  xTb[:, di, :], start=di == 0 and hp % 2 == 0,
                                  stop=di == 7 and hp % 2 == 1)
                  qT = sbq.tile([128, 8, 256], bf16, tag="qT", name=f"qT_{li}")
                  for t in range(2):
                      nc.vector.tensor_copy(
                          qT[:, t * 4:(t + 1) * 4, :],
                          ps_q[t][:, :].rearrange("p (h m) -> p h m", h=4))

                  # kT laid out key-chunk-major: [p, r(4), hp(8), m(256)]
                  kT = sbk.tile([128, 4, 8, 256], bf16, tag="kT", name=f"kT_{li}")
                  kn2 = sbk.tile([128, 8, 16, 65], bf16, tag="kn", name=f"kn2_{li}")
                  for r in range(4):
                      agq.dma_start(
                          out=kT[:, r, :, :],
                          in_=ago[r, 0, :, 0:2048].rearrange("p (h m) -> p h m", h=8))
                  km8 = sbs.tile([128, 4, 2], bf16, tag="km8", bufs=1,
                                 name=f"km8_{li}")
                  agq.dma_start(
                      out=km8[:, :, :],
                      in_=ago[:, 0, :, 2048:KCOLS].rearrange("r p m -> p r m"))
                  km8f = sbs.tile([128, 8], f32, tag="km8f", bufs=1,
                                  name=f"km8f_{li}")
                  nc.vector.tensor_copy(
                      km8f[:, :], km8[:, :, :].rearrange("p r m -> p (r m)"))
                  if TRUNC == "kread":
                      continue
                  NO_KN2 = os.environ.get("KERNEL_NO_KN2", "0") == "1"
                  # rebuild kn (+mask col) from gathered kT: kn_block = kT_block^T
                  # via regular matmul with an identity moving operand (f32 PSUM,
                  # per-bank start/stop chains)
                  # NB: all stationaries within one PSUM accumulation chain must
                  # share a base partition (HW constraint) — so even-sub heads
                  # chain into bank 0, odd-sub heads into bank 1.
                  for mi in range(0 if not NO_KN2 else 8, 8):
                      ps_n = pst(f"psn_{li}_{mi}")
                      for s in range(2):
                          lo = s * 64
                          for h2 in range(8):
                              nc.tensor.matmul(
                                  ps_n[:, s * 512 + h2 * 64:s * 512 + h2 * 64 + 64],
                                  kT[lo:lo + 64, mi // 2, h2,
                                     (mi % 2) * 128:(mi % 2) * 128 + 128],
                                  identb[lo:lo + 64, lo:lo + 64],
                                  start=h2 == 0, stop=h2 == 7)
                      nc.vector.tensor_scalar_mul(
                          kn2[:, mi, :, 0:64].rearrange(
                              "p (h2 s) c -> p h2 s c", s=2),
                          ps_n[:, :].rearrange(
                              "p (s h2 c) -> p h2 s c", s=2, h2=8),
                          km8f[:, mi:mi + 1])
                      nc.vector.tensor_scalar_mul(
                          kn2[:, mi, :, 64],
                          ones[:, 0:16].bitcast(f32),
                          km8f[:, mi:mi + 1])

                  if TRUNC == "q":
                      continue
                  mark(f"L{li}.attn")
                  # ---------------- attention, head by head ------------------
                  attnT = sba.tile([128, 8, 256], bf16, tag="attnT", bufs=1, name=f"attnT_{li}")
                  for h in range(H):
                      hp, sub = h // 2, h % 2
                      lo, hi = sub * 64, sub * 64 + 64
                      ps_s = [pst(f"pss_{li}_{h}_{t}") for t in range(2)]
                      for mi in range(8):
                          nc.tensor.matmul(
                              ps_s[mi // 4][:, (mi % 4) * 256:(mi % 4) * 256 + 256],
                              kT[lo:hi, mi // 2, hp, (mi % 2) * 128:(mi % 2) * 128 + 128],
                              qT[lo:hi, hp, :], start=True, stop=True)
                      wT = sba.tile([128, 8, 256], bf16, tag="wT", name=f"wT_{li}_{h}")
                      for w_ in range(2):
                          nc.scalar.activation(
                              wT[:, w_ * 4:(w_ + 1) * 4, :],
                              ps_s[w_][:, :].rearrange("p (i m) -> p i m", i=4),
                              AF.Exp, bias=0.0, scale=INV_SQ)
                      # numerator rows 0:64, denominator row 64 (mask column)
                      ps_o = pst(f"pso_{li}_{h}")
                      for mi in range(8):
                          st, sp = mi == 0, mi == 7
                          nc.tensor.matmul(
                              ps_o[0:65, 0:256],
                              kn2[:, mi, h, :],
                              wT[:, mi, :], start=st, stop=sp)
                      rcp = sbs.tile([1, 256], f32r, tag="rcp", bufs=2, name=f"rcp_{li}_{h}")
                      with nc.allow_low_precision(reason="f32r rounding ok"):
                          nc.vector.reciprocal(rcp[:, :], ps_o[64:65, 0:256])
                      nc.tensor.matmul(
                          ps_o[0:64, 512:768], ones[0:1, 0:64], rcp[:, :],
                          start=True, stop=True)
                      rep = sbs.tile([128, 256], f32, tag="rep", name=f"rep_{li}_{h}", bufs=2)
                      nc.vector.tensor_copy(rep[0:64, :], ps_o[0:64, 512:768])
                      nc.vector.tensor_mul(attnT[lo:hi, hp, :],
                                           ps_o[0:64, 0:256],
                                           rep[0:64, :])

                  if TRUNC == "attn":
                      continue
                  mark(f"L{li}.wo")
                  # ---------------- Wo + residual + LN1 ----------------------
                  ps_y = [pst(f"psy_{li}_{t}") for t in range(2)]
                  for hh in range(2):
                      wot = sbw.tile([128, 4, 1024], bf16, tag="w",
                                     name=f"wo_{li}_{hh}")
                      nc.sync.dma_start(
                          out=wot[:, :, :],
                          in_=wo_in[li, hh * 4:(hh + 1) * 4].rearrange("d p e -> p d e"))
                      for hj in range(4):
                          hp = hh * 4 + hj
                          for ei in range(8):
                              nc.tensor.matmul(
                                  ps_y[ei // 4][:, (ei % 4) * 256:(ei % 4) * 256 + 256],
                                  wot[:, hj, ei * 128:(ei + 1) * 128],
                                  attnT[:, hp, :], start=hp == 0 and ei % 2 == 0,
                                  stop=hp == 7 and ei % 2 == 1)
                  resid = sba.tile([128, 8, 256], f32r, tag="resid", bufs=1, name=f"res1_{li}")
                  for ei in range(8):
                      nc.vector.scalar_tensor_tensor(
                          out=resid[:, ei, :],
                          in0=ps_y[ei // 4][:, (ei % 4) * 256:(ei % 4) * 256 + 256],
                          scalar=0.0, in1=xT[:, ei, :].bitcast(f32),
                          op0=ALU.add, op1=ALU.add)

                  xT = sbx.tile([128, 8, LC], f32r, tag="xT", name=f"xT_{li}a")
                  xTb = sbx.tile([128, 8, LC], bf16, tag="xTb", name=f"xTb_{li}a")
                  layer_norm(resid, xT, xTb, lnt2, 0, f"{li}a")

                  if TRUNC == "wo":
                      continue
                  mark(f"L{li}.ffn1")
                  # ---------------- FFN --------------------------------------
                  hT = sbh.tile([128, 32, 256], bf16, tag="hT", name=f"hT_{li}")
                  b1c = sbs.tile([128, 32], f32, tag="b1c", name=f"b1c_{li}")
                  smq.dma_start(out=b1c[:, :], in_=b1c_in[li])
                  for jg in range(4):
                      ps_h = [pst(f"psh_{li}_{jg}_{t}") for t in range(2)]
                      for dh in range(4):
                          w1t = sbw.tile([128, 2, 1024], bf16, tag="wq4",
                                         name=f"w1_{li}_{jg}_{dh}")
                          nc.sync.dma_start(
                              out=w1t[:, :, :],
                              in_=w1_in[li, jg, dh * 2:(dh + 1) * 2].rearrange(
                                  "d p j -> p d j"))
                          for dj in range(2):
                              di = dh * 2 + dj
                              for jj in range(8):
                                  nc.tensor.matmul(
                                      ps_h[jj // 4][:, (jj % 4) * 256:(jj % 4) * 256 + 256],
                                      w1t[:, dj, jj * 128:(jj + 1) * 128],
                                      xTb[:, di, :], start=di == 0 and jj % 2 == 0,
                                      stop=di == 7 and jj % 2 == 1)
                      for jj in range(8):
                          nc.scalar.activation(
                              hT[:, jg * 8 + jj, :],
                              ps_h[jj // 4][:, (jj % 4) * 256:(jj % 4) * 256 + 256],
                              AF.Relu, bias=b1c[:, jg * 8 + jj:jg * 8 + jj + 1], scale=1.0)

                  if TRUNC == "ffn1":
                      continue
                  mark(f"L{li}.ffn2")
                  ps_f = [pst(f"psf_{li}_{t}") for t in range(2)]
                  for jg in range(4):
                      for ch in range(4):
                          w2t = sbw.tile([128, 2, 1024], bf16, tag="wq4",
                                         name=f"w2_{li}_{jg}_{ch}")
                          nc.sync.dma_start(
                              out=w2t[:, :, :],
                              in_=w2_in[li, jg, ch * 2:(ch + 1) * 2].rearrange(
                                  "c p e -> p c e"))
                          for cj in range(2):
                              jc = jg * 8 + ch * 2 + cj
                              for ei in range(8):
                                  nc.tensor.matmul(
                                      ps_f[ei // 4][:, (ei % 4) * 256:(ei % 4) * 256 + 256],
                                      w2t[:, cj, ei * 128:(ei + 1) * 128],
                                      hT[:, jc, :], start=jc == 0 and ei % 2 == 0,
                                      stop=jc == 31 and ei % 2 == 1)
                  b2c = sbs.tile([128, 8], f32, tag="b2c", name=f"b2c_{li}")
                  smq.dma_start(out=b2c[:, :], in_=b2c_in[li])
                  resid2 = sba.tile([128, 8, 256], f32r, tag="resid", bufs=1, name=f"res2_{li}")
                  for ei in range(8):
                      nc.vector.scalar_tensor_tensor(
                          out=resid2[:, ei, :],
                          in0=ps_f[ei // 4][:, (ei % 4) * 256:(ei % 4) * 256 + 256],
                          scalar=b2c[:, ei:ei + 1], in1=xT[:, ei, :].bitcast(f32),
                          op0=ALU.add, op1=ALU.add)

                  mark(f"L{li}.ln2")
                  xT = sbx.tile([128, 8, LC], f32r, tag="xT", name=f"xT_{li}b")
                  xTb = sbx.tile([128, 8, LC], bf16, tag="xTb", name=f"xTb_{li}b")
                  layer_norm(resid2, xT, xTb, lnt2, 1, f"{li}b")

            mark("out")
            # ---------------- output: transpose back --------------------
            for lj in range(2):
                outp = sbs.tile([128, 1024], f32, tag="outp", bufs=2,
                                name=f"outp_{lj}")
                for t in range(2):
                    ps_t = pst(f"pst_{lj}_{t}")
                    for ej in range(4):
                        ei = t * 4 + ej
                        nc.tensor.transpose(
                            ps_t[:, ej * 256:ej * 256 + 128],
                            xT[:, ei, lj * 128:(lj + 1) * 128].bitcast(f32),
                            ident[:, :])
                    nc.vector.tensor_copy(
                        outp[:, t * 512:(t + 1) * 512].rearrange(
                            "p (e m) -> p e m", e=4),
                        ps_t[:, :].rearrange("p (e m) -> p e m", e=4)[:, :, 0:128])
                nc.sync.dma_start(out=out_par[lj * 128:(lj + 1) * 128, :],
                                  in_=outp[:, :])
    nc.finalize()
    return nc


def _host_prep(inputs, n_layers):
    """Host-side preprocessing: embedding+PE, weight reshapes, per-core maps."""
    import ml_dtypes
    bf16 = ml_dtypes.bfloat16

    tokens = np.asarray(inputs["tokens"])
    mask = np.asarray(inputs["self_attn_mask"])
    emb = np.asarray(inputs["emb"], dtype=np.float32)
    Wq = np.asarray(inputs["Wq"], dtype=np.float32)
    Wk = np.asarray(inputs["Wk"], dtype=np.float32)
    Wo = np.asarray(inputs["Wo"], dtype=np.float32)
    W1 = np.asarray(inputs["W1"], dtype=np.float32)
    b1 = np.asarray(inputs["b1"], dtype=np.float32)
    W2 = np.asarray(inputs["W2"], dtype=np.float32)
    b2 = np.asarray(inputs["b2"], dtype=np.float32)
    g1 = np.asarray(inputs["ln1_g"], dtype=np.float32)
    be1 = np.asarray(inputs["ln1_b"], dtype=np.float32)
    g2 = np.asarray(inputs["ln2_g"], dtype=np.float32)
    be2 = np.asarray(inputs["ln2_b"], dtype=np.float32)

    # input block (exact f32, same ops as reference)
    emb0 = emb.copy()
    emb0[PAD] = 0.0
    x = emb0[tokens] * np.float32(D ** 0.5)
    pos = np.arange(L, dtype=np.float32)[:, None]
    i = np.arange(D // 2, dtype=np.float32)[None, :]
    angle = pos / (10000.0 ** (2.0 * i / D))
    pe = np.zeros((L, D), np.float32)
    pe[:, 0::2] = np.sin(angle)
    pe[:, 1::2] = np.cos(angle)
    x = (x + pe[None]).astype(np.float32)  # [B, L, D]

    # head-major permutation: new col r = h*64+d'  <- old col d'*H + h
    r = np.arange(D)
    perm = (r % HD) * H + (r // HD)
    Wq_p = np.ascontiguousarray(Wq[:n_layers][:, :, perm])
    Wk_p = np.ascontiguousarray(Wk[:n_layers][:, :, perm])
    Wo_p = np.ascontiguousarray(Wo[:n_layers][:, perm, :])

    wq_d = Wq_p.reshape(n_layers, 8, 128, 1024).astype(bf16)
    wk_d = Wk_p.reshape(n_layers, 8, 128, 1024).astype(bf16)
    wo_d = Wo_p.reshape(n_layers, 8, 128, 1024).astype(bf16)
    w1_d = np.ascontiguousarray(
        W1[:n_layers].reshape(n_layers, 8, 128, 4, 8, 128)
        .transpose(0, 3, 1, 2, 4, 5)).reshape(
        n_layers, 4, 8, 128, 1024).astype(bf16)
    w2_d = W2[:n_layers].reshape(n_layers, 4, 8, 128, 1024).astype(bf16)
    b1c = np.ascontiguousarray(
        b1[:n_layers].reshape(n_layers, 32, 128).transpose(0, 2, 1))
    b2c = np.ascontiguousarray(
        b2[:n_layers].reshape(n_layers, 8, 128).transpose(0, 2, 1))
    # lnp[li, p, k, ei]: k in (g1, b1, g2, b2); (ei, p) index D = ei*128+p
    lnp = np.stack([g1[:n_layers], be1[:n_layers], g2[:n_layers],
                    be2[:n_layers]], axis=1).reshape(n_layers, 4, 8, 128)
    lnp = np.ascontiguousarray(lnp.transpose(0, 3, 1, 2))
    ones = np.ones((128, 256), np.float32)

    shared = dict(wq=wq_d, wk=wk_d, wo=wo_d, w1=w1_d, w2=w2_d,
                  b1c=b1c, b2c=b2c, lnp=lnp, ones=ones)

    in_maps = []
    for c in range(NCORES):
        b = c // 4
        j = c % 4
        xT0 = np.ascontiguousarray(
            x[b, j * LC:(j + 1) * LC, :].T).reshape(8, 128, LC)
        keep = (~mask[b, 0, :, 0]).astype(np.float32)  # 1.0 where key kept
        kmo = np.ascontiguousarray(
            keep[j * LC:(j + 1) * LC].reshape(2, 128).T)
        m = dict(shared)
        m.update(xT_in=xT0, kmo=kmo)
        in_maps.append(m)
    return x, in_maps


def kernel(**inputs) -> np.ndarray:
    from concourse.bass_utils import run_bass_kernel_spmd

    n_layers = _DEV_NL
    if "nc" not in _cache or _cache.get("nl") != n_layers:
        _cache["nc"] = _build_nc(n_layers)
        _cache["nl"] = n_layers
    nc = _cache["nc"]

    _, in_maps = _host_prep(inputs, n_layers)
    res = run_bass_kernel_spmd(nc, in_maps, core_ids=list(range(NCORES)))
    out = np.empty((B, L, D), np.float32)
    for c in range(NCORES):
        b, j = c // 4, c % 4
        out[b, j * LC:(j + 1) * LC, :] = res.results[c]["out"]
    return out
